# revision 5
# baseline (speedup 1.0000x reference)
"""Biaffine labeler kernel for 8x Trainium2 NeuronCores.

Full-input contract: kernel(**inputs) takes the unsharded inputs and
returns the full [8, 256, 50] float32 logits.

Sharding: data-parallel over B — core i handles batch i. The projection
weights, bilinear tensor W, and biases are replicated (staged identically
for every core).

Per-core pipeline (T=256 tokens, D=1024, DL=512, NL=50 labels):
  1. dep_label  = dep[b] @ Wdep + bdep            (PE, bf16, K=1024)
  2. head_label = head[b] @ Whead                 (PE, bf16)
  3. one-hot(idx) built on DVE; sel^T = head_label^T gathered via a
     one-hot matmul on PE; bhead added on ACT during PSUM->SBUF copy
  4. P_n = sel @ W[n]^T for all 50 labels         (PE, bf16, 400 matmuls)
  5. logits[t,n] = bias[n] + sum_d dep_label[t,d] * P_n[t,d]
                                                  (DVE tensor_tensor_reduce)
W streams from HBM in bf16 as label-group chunks (4 labels = 2 MB per
DMA), double-buffered against the PE main loop.
"""

import sys

sys.path.insert(0, "/opt/trn_rl_repo")

import numpy as np
import ml_dtypes

B, T, D = 8, 256, 1024
NL, DL = 50, 512
GROUP = 4  # labels per PSUM group (x2 token tiles = 8 PSUM banks)
N_GROUPS = (NL + GROUP - 1) // GROUP  # 13
GF = GROUP * 4 * DL  # free-dim elems per full W group chunk (8192)

BF16 = ml_dtypes.bfloat16

# Stash of the last run's BassKernelResults (exec_time_ns when BASS_TRACE=1).
LAST_RESULTS = None

_NC_CACHE = None


def _build_nc():
    import concourse.bass as bass  # noqa: F401
    import concourse.bacc as bacc
    import concourse.mybir as mybir
    import concourse.tile as tile

    bf = mybir.dt.bfloat16
    f32 = mybir.dt.float32
    Alu = mybir.AluOpType
    Act = mybir.ActivationFunctionType

    nc = bacc.Bacc(None)

    # --- DRAM I/O ---------------------------------------------------------
    depT = nc.dram_tensor("depT", [128, 2048], bf, kind="ExternalInput")
    headT = nc.dram_tensor("headT", [128, 2048], bf, kind="ExternalInput")
    wdep = nc.dram_tensor("wdep", [128, 4096], bf, kind="ExternalInput")
    whead = nc.dram_tensor("whead", [128, 4096], bf, kind="ExternalInput")
    wg = nc.dram_tensor("wg", [N_GROUPS, 128, GF], bf, kind="ExternalInput")
    idxb = nc.dram_tensor("idxb", [128, 256], f32, kind="ExternalInput")
    iota2 = nc.dram_tensor("iota2", [128, 2], f32, kind="ExternalInput")
    bheadc = nc.dram_tensor("bheadc", [128, 4], f32, kind="ExternalInput")
    bdepr = nc.dram_tensor("bdepr", [128, 512], f32, kind="ExternalInput")
    biasr = nc.dram_tensor("biasr", [128, 64], f32, kind="ExternalInput")
    out = nc.dram_tensor("out", [256, 64], f32, kind="ExternalOutput")

    groups = [list(range(g * GROUP, min((g + 1) * GROUP, NL))) for g in range(N_GROUPS)]

    with tile.TileContext(nc) as tc:
        with (
            tc.sbuf_pool(name="cpool", bufs=1) as cpool,
            tc.sbuf_pool(name="persist", bufs=1) as pers,
            tc.sbuf_pool(name="wpool", bufs=3) as wpool,
            tc.sbuf_pool(name="spool", bufs=4) as spool,
            tc.psum_pool(name="ps", bufs=8) as ps,
        ):
            # --- constants / per-core activations in ----------------------
            depT_sb = cpool.tile([128, 2048], bf)
            nc.sync.dma_start(depT_sb[:], depT[:])
            headT_sb = cpool.tile([128, 2048], bf)
            nc.sync.dma_start(headT_sb[:], headT[:])
            wdep_sb = cpool.tile([128, 4096], bf)
            nc.sync.dma_start(wdep_sb[:], wdep[:])
            whead_sb = cpool.tile([128, 4096], bf)
            nc.sync.dma_start(whead_sb[:], whead[:])
            idxb_sb = cpool.tile([128, 256], f32)
            nc.sync.dma_start(idxb_sb[:], idxb[:])
            iota2_sb = cpool.tile([128, 2], f32)
            nc.sync.dma_start(iota2_sb[:], iota2[:])
            bheadc_sb = cpool.tile([128, 4], f32)
            nc.sync.dma_start(bheadc_sb[:], bheadc[:])
            bdepr_sb = cpool.tile([128, 512], f32)
            nc.sync.dma_start(bdepr_sb[:], bdepr[:])
            biasr_sb = cpool.tile([128, 64], f32)
            nc.sync.dma_start(biasr_sb[:], biasr[:])

            # --- dep projection: dep_label[m] = [128 t, 512 d] f32 --------
            dep_label = []
            for m in range(2):
                pd = ps.tile([128, 512], f32, tag="ps")
                for k in range(8):
                    nc.tensor.matmul(
                        pd[:],
                        lhsT=depT_sb[:, k * 256 + m * 128 : k * 256 + m * 128 + 128],
                        rhs=wdep_sb[:, k * 512 : (k + 1) * 512],
                        start=(k == 0),
                        stop=(k == 7),
                    )
                dl = pers.tile([128, 512], f32, tag=f"dl{m}")
                nc.vector.tensor_tensor(dl[:], pd[:], bdepr_sb[:], Alu.add)
                dep_label.append(dl)

            # --- head projection: head_label[j] = [128 t, 512 d] bf16 -----
            head_label = []
            for j in range(2):
                ph = ps.tile([128, 512], f32, tag="ps")
                for k in range(8):
                    nc.tensor.matmul(
                        ph[:],
                        lhsT=headT_sb[:, k * 256 + j * 128 : k * 256 + j * 128 + 128],
                        rhs=whead_sb[:, k * 512 : (k + 1) * 512],
                        start=(k == 0),
                        stop=(k == 7),
                    )
                hlj = pers.tile([128, 512], bf, tag=f"hl{j}")
                nc.scalar.copy(hlj[:], ph[:])
                head_label.append(hlj)

            # --- one-hot of head_indices: oh[j][p, t] = (idx[t] == 128j+p)
            onehot = []
            for j in range(2):
                ohj = pers.tile([128, 256], bf, tag=f"oh{j}")
                nc.vector.tensor_scalar(
                    out=ohj[:],
                    in0=idxb_sb[:],
                    scalar1=iota2_sb[:, j : j + 1],
                    scalar2=None,
                    op0=Alu.is_equal,
                )
                onehot.append(ohj)

            # --- gather: selT[c] = [128 e, 256 t] bf16 (+bhead on copy) ---
            selT = []
            for c in range(4):
                pg = ps.tile([128, 256], f32, tag="ps")
                for j in range(2):
                    nc.tensor.matmul(
                        pg[:],
                        lhsT=head_label[j][:, c * 128 : (c + 1) * 128],
                        rhs=onehot[j][:],
                        start=(j == 0),
                        stop=(j == 1),
                    )
                sc = pers.tile([128, 256], bf, tag=f"sel{c}")
                nc.scalar.activation(
                    sc[:], pg[:], Act.Identity, bias=bheadc_sb[:, c : c + 1], scale=1.0
                )
                selT.append(sc)

            # --- output accumulators --------------------------------------
            out_sb = []
            for m in range(2):
                om = pers.tile([128, 64], f32, tag=f"out{m}")
                out_sb.append(om)

            # --- main biaffine loop over label groups ---------------------
            for g, labels in enumerate(groups):
                ng = len(labels)
                wg_sb = wpool.tile([128, GF], bf, tag="wg")
                nc.sync.dma_start(wg_sb[:, : ng * 4 * 512], wg[g, :, : ng * 4 * 512])

                pb = {}
                for li in range(ng):
                    for m in range(2):
                        pb[li, m] = ps.tile(
                            [128, 512], f32, tag="ps", name=f"pb_{g}_{li}_{m}"
                        )
                for k in range(4):
                    for m in range(2):
                        for li in range(ng):
                            nc.tensor.matmul(
                                pb[li, m][:],
                                lhsT=selT[k][:, m * 128 : (m + 1) * 128],
                                rhs=wg_sb[:, (li * 4 + k) * 512 : (li * 4 + k + 1) * 512],
                                start=(k == 0),
                                stop=(k == 3),
                            )
                for li, n in enumerate(labels):
                    for m in range(2):
                        prod = spool.tile([128, 512], f32, tag="prod", name="prod")
                        nc.vector.scalar_tensor_tensor(
                            out=prod[:],
                            in0=pb[li, m][:],
                            scalar=1.0,
                            in1=dep_label[m][:],
                            op0=Alu.mult,
                            op1=Alu.mult,
                            accum_out=out_sb[m][:, n : n + 1],
                        )

            # --- add label bias, store ------------------------------------
            for m in range(2):
                fin = pers.tile([128, 64], f32, tag=f"fin{m}", name=f"fin{m}")
                nc.vector.tensor_tensor(
                    fin[:, :NL], out_sb[m][:, :NL], biasr_sb[:, :NL], Alu.add
                )
                nc.sync.dma_start(out[m * 128 : (m + 1) * 128, :NL], fin[:, :NL])

    nc.finalize()
    return nc


def _stage_shared(Wdep, bdep, Whead, bhead, W, bias):
    """Host-side staging of the replicated tensors."""
    wdep_h = np.ascontiguousarray(
        Wdep.reshape(8, 128, 512).transpose(1, 0, 2).reshape(128, 4096)
    ).astype(BF16)
    whead_h = np.ascontiguousarray(
        Whead.reshape(8, 128, 512).transpose(1, 0, 2).reshape(128, 4096)
    ).astype(BF16)

    # W[n, d, e] -> WT[n, k, p, d] = W[n, d, k*128+p]
    WT = np.ascontiguousarray(W.transpose(0, 2, 1)).reshape(NL, 4, 128, 512)
    wg_h = np.zeros((N_GROUPS, 128, GF), dtype=BF16)
    for g in range(N_GROUPS):
        labels = list(range(g * GROUP, min((g + 1) * GROUP, NL)))
        blk = WT[labels]  # [ng, 4, 128, 512]
        ng = len(labels)
        wg_h[g, :, : ng * 4 * 512] = (
            blk.transpose(2, 0, 1, 3).reshape(128, ng * 4 * 512).astype(BF16)
        )

    iota2_h = np.stack(
        [np.arange(128, dtype=np.float32), 128 + np.arange(128, dtype=np.float32)],
        axis=1,
    )
    bheadc_h = np.ascontiguousarray(bhead.reshape(4, 128).T).astype(np.float32)
    bdepr_h = np.ascontiguousarray(
        np.broadcast_to(bdep[None, :], (128, 512))
    ).astype(np.float32)
    biasr_h = np.zeros((128, 64), dtype=np.float32)
    biasr_h[:, :NL] = bias[None, :]

    return {
        "wdep": wdep_h,
        "whead": whead_h,
        "wg": wg_h,
        "iota2": iota2_h,
        "bheadc": bheadc_h,
        "bdepr": bdepr_h,
        "biasr": biasr_h,
    }


def _stage_core(dep_b, head_b, idx_b):
    """Host-side staging of one batch's activations."""
    depT_h = np.ascontiguousarray(
        dep_b.T.reshape(8, 128, 256).transpose(1, 0, 2).reshape(128, 2048)
    ).astype(BF16)
    headT_h = np.ascontiguousarray(
        head_b.T.reshape(8, 128, 256).transpose(1, 0, 2).reshape(128, 2048)
    ).astype(BF16)
    idxb_h = np.ascontiguousarray(
        np.broadcast_to(idx_b.astype(np.float32)[None, :], (128, 256))
    )
    return {"depT": depT_h, "headT": headT_h, "idxb": idxb_h}


def kernel(dep, head, head_indices, mask, Wdep, bdep, Whead, bhead, W, bias):
    global LAST_RESULTS, _NC_CACHE
    from concourse.bass_utils import run_bass_kernel_spmd

    dep = np.asarray(dep, dtype=np.float32)
    head = np.asarray(head, dtype=np.float32)
    head_indices = np.asarray(head_indices)
    Wdep = np.asarray(Wdep, dtype=np.float32)
    bdep = np.asarray(bdep, dtype=np.float32)
    Whead = np.asarray(Whead, dtype=np.float32)
    bhead = np.asarray(bhead, dtype=np.float32)
    W = np.asarray(W, dtype=np.float32)
    bias = np.asarray(bias, dtype=np.float32)

    if _NC_CACHE is None:
        _NC_CACHE = _build_nc()
    nc = _NC_CACHE

    shared = _stage_shared(Wdep, bdep, Whead, bhead, W, bias)
    in_maps = []
    for b in range(B):
        m = dict(shared)
        m.update(_stage_core(dep[b], head[b], head_indices[b]))
        in_maps.append(m)

    res = run_bass_kernel_spmd(nc, in_maps, list(range(B)))
    LAST_RESULTS = res
    outs = [np.asarray(res.results[b]["out"][:, :NL], dtype=np.float32) for b in range(B)]
    return np.stack(outs, axis=0)
